# revision 18
# baseline (speedup 1.0000x reference)
"""Trainium2 Bass kernel for nn_DeslicingDecoder (moe_routing).

Contract: kernel(**inputs) takes FULL unsharded numpy inputs (as produced by
setup_inputs()) and returns the FULL output tuple matching reference():
  (z_out[N,256] f32, prob_bin[N,1] f32, logits[N,11] f32, pred_large[N,1] f32,
   mask_bin[N] bool, mask_int_small[N] bool, mask_int_large[N] bool,
   offsets[N] f32, ranges[N] int32)

Sharding: N=30000 variables split contiguously across 8 NeuronCores (3750
rows each, padded to 4096).  The per-graph token tensor is replicated via a
host-precomputed projection; the deslice gather is turned into a dense
matmul per 256-token group against a per-group "augmented" attention matrix
(columns blocked by the <=KB batches the group spans), so all shapes are
static regardless of where batch boundaries fall.

Device pipeline per 256-token group (2 subtiles of 128 tokens):
  deslice matmul (fp32r) -> +z_var_0 (DVE) -> LayerNorm1 stats (bn_stats)
  -> batched Newton rsqrt (DVE) -> z_out (Pool) -> z_ln2 bf16 (Pool)
  -> DMA-transpose to feature-major -> W1 (bf16) -> Gelu (ACT, one op per
  layer over a 3-bank PSUM tile) -> W2 (bf16) -> Gelu -> head matmuls with
  the residual folded in via PSUM accumulation + per-token rescale (DVE)
  -> logit masking (DVE select).  Routing masks/offsets/ranges are computed
  once per core on [128,32]-packed tiles.
"""

import functools
from contextlib import ExitStack

import ml_dtypes
import numpy as np

import concourse.bass as bass
import concourse.tile as tile
from concourse import bacc, masks, mybir
from concourse.alu_op_type import AluOpType as ALU
from concourse.bass_utils import run_bass_kernel_spmd

F32 = mybir.dt.float32
F32R = mybir.dt.float32r
BF16 = mybir.dt.bfloat16
I32 = mybir.dt.int32
AF = mybir.ActivationFunctionType

# problem constants (hardcoded per contract)
N, K, B, D, T = 30000, 32, 8, 256, 10
NC_CLS = T + 1          # 11
LB_COL, UB_COL = 21, 22
INF = 1e18

NCORES = 8
NPC = N // NCORES       # 3750 rows per core
NPAD = 3840             # padded rows per core
TOK = 256               # tokens per group
NSUB = TOK // 128       # 2 subtiles per group
NGRP = NPAD // TOK      # 15 groups
NT = NPAD // 128        # 30 subtile columns

P = 128
HEAD_COLS = ((0, 1), (1, 12), (12, 13))   # bin, ints, intl columns in y


# --------------------------------------------------------------------------
# device program
# --------------------------------------------------------------------------

def _build_program(kb32: int, use_tanh_for_sim: bool = False):
    """Build the Bass/Tile program.  kb32 = 32 * max batches per group."""
    gelu_f = AF.Tanh if use_tanh_for_sim else AF.Gelu

    nc = bacc.Bacc("TRN2", target_bir_lowering=False, debug=False)

    # ---- dram io
    in2_d = nc.dram_tensor("in2", [NGRP, kb32, TOK + D], BF16, kind="ExternalInput").ap()
    zv0_d = nc.dram_tensor("zv0", [NPAD, D], F32, kind="ExternalInput").ap()
    vt_d = nc.dram_tensor("vt", [P, NT], F32, kind="ExternalInput").ap()
    lb_d = nc.dram_tensor("lb", [P, NT], F32, kind="ExternalInput").ap()
    ub_d = nc.dram_tensor("ub", [P, NT], F32, kind="ExternalInput").ap()
    w1_d = nc.dram_tensor("w1t", [2, P, 768], BF16, kind="ExternalInput").ap()
    w2_d = nc.dram_tensor("w2t", [2, P, 768], BF16, kind="ExternalInput").ap()
    wh_d = nc.dram_tensor("wht", [2, P, 16], BF16, kind="ExternalInput").ap()
    bhb_d = nc.dram_tensor("bhb", [1, 1], F32, kind="ExternalInput").ap()  # 0.5*bh_bin

    oz_d = nc.dram_tensor("oz", [NPAD, D], F32, kind="ExternalOutput").ap()
    olog_d = nc.dram_tensor("olog", [P, NT * NC_CLS], F32, kind="ExternalOutput").ap()
    oprob_d = nc.dram_tensor("oprob", [P, NT], F32, kind="ExternalOutput").ap()
    opred_d = nc.dram_tensor("opred", [P, NT], F32, kind="ExternalOutput").ap()
    omb_d = nc.dram_tensor("omb", [P, NT], F32, kind="ExternalOutput").ap()
    oms_d = nc.dram_tensor("oms", [P, NT], F32, kind="ExternalOutput").ap()
    oml_d = nc.dram_tensor("oml", [P, NT], F32, kind="ExternalOutput").ap()
    ooff_d = nc.dram_tensor("ooff", [P, NT], F32, kind="ExternalOutput").ap()
    orng_d = nc.dram_tensor("orng", [P, NT], I32, kind="ExternalOutput").ap()

    with tile.TileContext(nc) as tc, ExitStack() as ctx:
        const = ctx.enter_context(tc.tile_pool(name="const", bufs=1))
        io = ctx.enter_context(tc.tile_pool(name="io", bufs=4))
        zw = ctx.enter_context(tc.tile_pool(name="zw", bufs=NGRP))
        zo = ctx.enter_context(tc.tile_pool(name="zo", bufs=3))
        zl = ctx.enter_context(tc.tile_pool(name="zl", bufs=3))
        gp = ctx.enter_context(tc.tile_pool(name="gp", bufs=2))
        stp = ctx.enter_context(tc.tile_pool(name="stp", bufs=3))
        rt = ctx.enter_context(tc.tile_pool(name="rt", bufs=1))
        ps_small = ctx.enter_context(tc.tile_pool(name="ps_small", bufs=2, space="PSUM"))
        ps_h = ctx.enter_context(tc.tile_pool(name="ps_h", bufs=2, space="PSUM"))

        # ---- constants / weights
        w1t = const.tile([P, 2, 768], BF16)
        nc.sync.dma_start(w1t, w1_d.rearrange("k p f -> p k f"))
        w2t = const.tile([P, 2, 768], BF16)
        nc.sync.dma_start(w2t, w2_d.rearrange("k p f -> p k f"))
        wht = const.tile([P, 2, 16], BF16)
        nc.sync.dma_start(wht, wh_d.rearrange("k p f -> p k f"))
        bhb = const.tile([P, 1], F32)
        nc.sync.dma_start(bhb, bhb_d.to_broadcast((P, 1)))

        idb = const.tile([P, P], BF16)
        masks.make_identity(nc, idb)

        iota_i = const.tile([P, NC_CLS], I32)
        nc.gpsimd.iota(iota_i, [[1, NC_CLS]], channel_multiplier=0)
        iota_f = const.tile([P, NC_CLS], F32)
        nc.vector.tensor_copy(iota_f, iota_i)
        neg9 = const.tile([P, NC_CLS], F32)
        nc.vector.memset(neg9, -1e9)

        # ---- routing (once per core, [128, NT] col-major packed)
        vt_t = rt.tile([P, NT], F32)
        nc.sync.dma_start(vt_t, vt_d)
        lb_t = rt.tile([P, NT], F32)
        nc.sync.dma_start(lb_t, lb_d)
        ub_t = rt.tile([P, NT], F32)
        nc.sync.dma_start(ub_t, ub_d)

        mb = rt.tile([P, NT], F32)
        nc.vector.tensor_scalar(mb, vt_t, 1.0, None, ALU.is_equal)
        nc.sync.dma_start(omb_d, mb)
        isint = rt.tile([P, NT], F32)
        nc.vector.tensor_scalar(isint, vt_t, 2.0, None, ALU.is_equal)
        fin = rt.tile([P, NT], F32)
        nc.vector.tensor_tensor(fin, lb_t, lb_t, ALU.mult)
        nc.vector.tensor_scalar(fin, fin, INF * INF, None, ALU.is_lt)
        fu = rt.tile([P, NT], F32)
        nc.vector.tensor_tensor(fu, ub_t, ub_t, ALU.mult)
        nc.vector.tensor_scalar(fu, fu, INF * INF, None, ALU.is_lt)
        nc.vector.tensor_tensor(fin, fin, fu, ALU.mult)
        dd = rt.tile([P, NT], F32)
        nc.vector.tensor_tensor(dd, ub_t, lb_t, ALU.subtract)
        sle = rt.tile([P, NT], F32)
        nc.vector.tensor_scalar(sle, dd, float(T), None, ALU.is_le)
        sm = rt.tile([P, NT], F32)
        nc.vector.tensor_tensor(sm, isint, fin, ALU.mult)
        nc.vector.tensor_tensor(sm, sm, sle, ALU.mult)
        nc.sync.dma_start(oms_d, sm)
        lg = rt.tile([P, NT], F32)
        nc.vector.tensor_tensor(lg, isint, sm, ALU.subtract)
        nc.sync.dma_start(oml_d, lg)
        nc.sync.dma_start(ooff_d, lb_t)     # offsets = floor(lb) = lb (integral)
        rngf = rt.tile([P, NT], F32)
        nc.vector.tensor_scalar(rngf, dd, 1.0, float(NC_CLS), ALU.add, ALU.min)
        nc.vector.tensor_scalar(rngf, rngf, 1.0, None, ALU.max)
        rngi = rt.tile([P, NT], I32)
        nc.vector.tensor_copy(rngi, rngf)
        nc.sync.dma_start(orng_d, rngi)

        y_all = rt.tile([P, NT, 16], F32)
        logits_buf = rt.tile([P, NT, NC_CLS], F32)
        msk3 = rt.tile([P, NT, NC_CLS], mybir.dt.uint8)
        i_b = bass.AP(tensor=iota_f.tensor, offset=iota_f.offset,
                      ap=[iota_f.ap[0], [0, NT], [1, NC_CLS]])
        r_b = bass.AP(tensor=rngf.tensor, offset=rngf.offset,
                      ap=[rngf.ap[0], [1, NT], [0, NC_CLS]])
        nc.vector.tensor_tensor(msk3, i_b, r_b, ALU.is_lt)

        # ---- phase A: deslice + z_sum + LN1 stats for all groups
        mv = rt.tile([P, NT, 2], F32)
        zs_list = []
        for g in range(NGRP):
            in2 = io.tile([kb32, TOK + D], BF16)
            nc.scalar.dma_start(in2, in2_d[g])
            zv0 = io.tile([P, NSUB, D], F32)
            nc.scalar.dma_start(
                zv0, zv0_d[g * TOK:(g + 1) * TOK, :].rearrange(
                    "(p s) d -> p s d", s=NSUB))
            zdes = ps_small.tile([P, NSUB, D], F32, tag="sp")
            attn_ap = in2[:, 0:TOK]
            tok_ap = in2[:, TOK:TOK + D]
            for s in range(NSUB):
                nc.tensor.matmul(
                    zdes[:, s, :], attn_ap[:, s * P:(s + 1) * P], tok_ap,
                    start=True, stop=True)
            zs = zw.tile([P, NSUB, D], F32)
            nc.vector.tensor_tensor(zs, zdes, zv0, ALU.add)
            st = stp.tile([P, NSUB, 6], F32)
            for s in range(NSUB):
                nc.vector.bn_stats(st[:, s, :], zs[:, s, :])
                nc.vector.bn_aggr(mv[:, g * NSUB + s, :], st[:, s, :])
            zs_list.append(zs)

        # ---- phase N: batched Newton rsqrt over all 30 subtiles
        # y1 = rsqrt(var1+eps); q = rsqrt(var2+eps) with var2 = var1*y1^2
        # (fusion norm is identity for this input family); inv2 = 1/q.
        vv = rt.tile([P, NT], F32)
        nc.vector.tensor_scalar(vv, mv[:, :, 1], 1e-5, None, ALU.add)
        y1 = rt.tile([P, NT], F32)
        nc.vector.memset(y1, 0.7)
        tq = rt.tile([P, NT], F32)
        for _ in range(4):
            nc.vector.tensor_tensor(tq, y1, y1, ALU.mult)
            nc.vector.tensor_tensor(tq, tq, vv, ALU.mult)
            nc.vector.tensor_scalar(tq, tq, -0.5, 1.5, ALU.mult, ALU.add)
            nc.vector.tensor_tensor(y1, y1, tq, ALU.mult)
        v2p = rt.tile([P, NT], F32)
        nc.vector.tensor_tensor(v2p, y1, y1, ALU.mult)
        nc.vector.tensor_tensor(v2p, v2p, mv[:, :, 1], ALU.mult)
        nc.vector.tensor_scalar(v2p, v2p, 1e-5, None, ALU.add)
        q = rt.tile([P, NT], F32)
        nc.vector.memset(q, 1.0)
        for _ in range(2):
            nc.vector.tensor_tensor(tq, q, q, ALU.mult)
            nc.vector.tensor_tensor(tq, tq, v2p, ALU.mult)
            nc.vector.tensor_scalar(tq, tq, -0.5, 1.5, ALU.mult, ALU.add)
            nc.vector.tensor_tensor(q, q, tq, ALU.mult)
        inv2 = rt.tile([P, NT], F32)
        nc.vector.tensor_tensor(inv2, v2p, q, ALU.mult)   # sqrt(v2p) = 1/q
        rc = rt.tile([P, NT], F32)
        nc.vector.tensor_tensor(rc, y1, q, ALU.mult)      # rstd1 * rstd2

        # ---- phase B: LN apply + MLP heads per group
        def bc2(tile2d, g, width):
            # [128, 2] slice (cols 2g, 2g+1) broadcast along a width-free dim
            return bass.AP(tensor=tile2d.tensor, offset=tile2d.offset + 2 * g,
                           ap=[tile2d.ap[0], [1, 2], [0, width]])

        for g in range(NGRP):
            zs = zs_list[g]
            tln = zl.tile([P, NSUB, D], F32, tag="tln")
            m_bc = bass.AP(tensor=mv.tensor, offset=mv.offset + 4 * g,
                           ap=[mv.ap[0], [2, 2], [0, D]])
            nc.vector.tensor_tensor(tln, zs, m_bc, ALU.subtract)
            zout = zo.tile([P, NSUB, D], F32)
            nc.vector.tensor_tensor(zout, tln, bc2(y1, g, D), ALU.mult)
            zln2 = zl.tile([P, NSUB, D], BF16)
            nc.vector.tensor_tensor(zln2, tln, bc2(rc, g, D), ALU.mult)
            nc.sync.dma_start(
                oz_d[g * TOK:(g + 1) * TOK, :].rearrange(
                    "(p s) d -> p s d", s=NSUB), zout)

            # transpose z_ln2 to feature-major via PE + one DVE copy
            zt_ps = ps_small.tile([P, 2, NSUB, P], BF16, tag="sp")
            for k in range(2):
                for s_i in range(NSUB):
                    nc.tensor.transpose(
                        zt_ps[:, k, s_i, :], zln2[:, s_i, k * P:(k + 1) * P], idb)
            zts = gp.tile([P, 2, NSUB, P], BF16, tag="zts")
            nc.vector.tensor_copy(zts, zt_ps)

            # W1: feature-major h1T [6*128 hid, TOK]
            h1 = ps_h.tile([P, 6, TOK], F32, tag="h")
            for c in range(6):
                for k in range(2):
                    nc.tensor.matmul(
                        h1[:, c, :],
                        w1t[:, k, c * P:(c + 1) * P],
                        zts[:, k, :, :],
                        start=(k == 0), stop=(k == 1))
            g1 = gp.tile([P, 6, TOK], BF16)
            nc.scalar.activation(g1, h1, gelu_f)

            # W2 per head: h2T chunks (head h, dout chunk j)
            h2 = ps_h.tile([P, 6, TOK], F32, tag="h")
            for h in range(3):
                for j in range(2):
                    for k in range(2):
                        nc.tensor.matmul(
                            h2[:, h * 2 + j, :],
                            w2t[:, k, h * 256 + j * P:h * 256 + (j + 1) * P],
                            g1[:, h * 2 + k, :],
                            start=(k == 0), stop=(k == 1))
            g2 = gp.tile([P, 6, TOK], BF16)
            nc.scalar.activation(g2, h2, gelu_f)

            # heads: y = inv2*(z_ln2 @ WheT) + g2 @ WheT   (residual folded)
            yps = ps_small.tile([P, NSUB, 2, 16], F32, tag="sp")
            for s_i in range(NSUB):
                for k in range(2):
                    nc.tensor.matmul(
                        yps[:, s_i, 0, 0:13], zts[:, k, s_i, :], wht[:, k, 0:13],
                        start=(k == 0), stop=(k == 1))
                for h, (o0, o1) in enumerate(HEAD_COLS):
                    for k in range(2):
                        nc.tensor.matmul(
                            yps[:, s_i, 1, o0:o1],
                            g2[:, h * 2 + k, s_i * P:(s_i + 1) * P],
                            wht[:, k, o0:o1],
                            start=(k == 0), stop=(k == 1))
            ya = yps[:, :, 0, 0:13]
            yb = yps[:, :, 1, 0:13]
            ysl = bass.AP(tensor=y_all.tensor, offset=y_all.offset + 32 * g,
                          ap=[y_all.ap[0], [16, 2], [1, 13]])
            nc.vector.tensor_tensor(ysl, ya, bc2(inv2, g, 13), ALU.mult)
            nc.vector.tensor_tensor(ysl, ysl, yb, ALU.add)
            for s_i in range(NSUB):
                c = g * NSUB + s_i
                nc.vector.select(
                    logits_buf[:, c, :], msk3[:, c, :],
                    y_all[:, c, 1:12], neg9)

        # ---- final outputs
        nc.sync.dma_start(
            olog_d.rearrange("p (j o) -> p j o", o=NC_CLS), logits_buf)
        tanh_t = rt.tile([P, NT], F32)
        nc.scalar.activation(tanh_t, y_all[:, :, 0], AF.Tanh, bias=bhb, scale=0.5)
        prob_t = rt.tile([P, NT], F32)
        nc.vector.tensor_scalar(prob_t, tanh_t, 0.5, 0.5, ALU.mult, ALU.add)
        nc.sync.dma_start(oprob_d, prob_t)
        nc.sync.dma_start(opred_d, y_all[:, :, 12])

    nc.compile()
    return nc


@functools.lru_cache(maxsize=4)
def _get_program(kb32: int, use_tanh_for_sim: bool = False):
    return _build_program(kb32, use_tanh_for_sim)


# --------------------------------------------------------------------------
# host side
# --------------------------------------------------------------------------

def _np_gelu(x):
    from scipy.special import erf
    return 0.5 * x * (1.0 + erf(x / np.sqrt(2.0).astype(np.float32)))


def _np_ln(x, g, b, eps=1e-5):
    m = x.mean(-1, keepdims=True)
    v = ((x - m) ** 2).mean(-1, keepdims=True)
    return (x - m) / np.sqrt(v + eps) * g + b


def _np_head(x, ng, nb, W1, b1, W2, b2, Wh, bh):
    h = _np_ln(x, ng, nb)
    h = _np_gelu(h @ W1.T + b1)
    h = _np_gelu(h @ W2.T + b2)
    return (x + h) @ Wh.T + bh


def _host_fallback(inp):
    """Pure numpy reference path (used only for inputs outside the fast
    path's host-verified assumptions)."""
    et = inp['evolved_tokens'].astype(np.float32)
    proj = et @ inp['Wp'].T.astype(np.float32) + inp['bp']
    tok3 = proj.reshape(B, K, D)
    vb = np.asarray(inp['var_batch']).astype(np.int64)
    aw = inp['attn_weights'].astype(np.float32)
    z_des = np.einsum('nk,nkd->nd', aw, tok3[vb]).astype(np.float32)
    z_out = _np_ln(z_des + inp['z_var_0'], inp['fn_g'], inp['fn_b']).astype(np.float32)
    vf = inp['variable_features']
    lb = vf[:, LB_COL]
    ub = vf[:, UB_COL]
    vt = np.asarray(inp['var_types'])
    is_int = vt == 2
    finite = (np.abs(lb) < INF) & (np.abs(ub) < INF)
    m_small = is_int & finite & ((ub - lb) <= T)
    m_large = is_int & ~m_small
    m_bin = vt == 1
    offsets = np.floor(lb).astype(np.float32)
    ranges = np.clip(np.ceil(ub) - offsets + 1.0, 1.0, NC_CLS).astype(np.int32)
    prob_bin = 1.0 / (1.0 + np.exp(-_np_head(
        z_out, inp['bin_ng'], inp['bin_nb'], inp['bin_W1'], inp['bin_b1'],
        inp['bin_W2'], inp['bin_b2'], inp['bin_Wh'], inp['bin_bh'])))
    logits = _np_head(
        z_out, inp['ints_ng'], inp['ints_nb'], inp['ints_W1'], inp['ints_b1'],
        inp['ints_W2'], inp['ints_b2'], inp['ints_Wh'], inp['ints_bh'])
    valid = np.arange(NC_CLS)[None, :] < ranges[:, None]
    logits = np.where(valid, logits, np.float32(-1e9)).astype(np.float32)
    pred_large = _np_head(
        z_out, inp['intl_ng'], inp['intl_nb'], inp['intl_W1'], inp['intl_b1'],
        inp['intl_W2'], inp['intl_b2'], inp['intl_Wh'], inp['intl_bh'])
    return (z_out.astype(np.float32), prob_bin.astype(np.float32),
            logits, pred_large.astype(np.float32),
            m_bin, m_small, m_large, offsets, ranges)


def _pack_cols(x):
    """[NPAD] -> [128, NT]; packed[p, 2g+s] = x[g*256 + 2p + s]."""
    return np.ascontiguousarray(
        x.reshape(NGRP, P, NSUB).transpose(1, 0, 2).reshape(P, NT))


def _unpack_cols(x):
    return np.ascontiguousarray(
        x.reshape(P, NGRP, NSUB).transpose(1, 0, 2).reshape(-1))


def _prep_inputs(inp):
    """Build per-core device input maps.  Returns (in_maps, kb32) or None if
    the inputs fall outside the fast path's assumptions."""
    f32 = np.float32
    bf16 = ml_dtypes.bfloat16

    fn_g = np.asarray(inp['fn_g'], f32)
    fn_b = np.asarray(inp['fn_b'], f32)
    if not (np.all(fn_g == 1.0) and np.all(fn_b == 0.0)):
        return None
    vf = np.asarray(inp['variable_features'], f32)
    lb = vf[:, LB_COL].astype(f32)
    ub = vf[:, UB_COL].astype(f32)
    if not (np.all(np.floor(lb) == lb) and np.all(np.ceil(ub) == ub)):
        return None

    # fold per-head norm gains into W1, check biases are zero
    w1e = []
    for name in ('bin', 'ints', 'intl'):
        ng = np.asarray(inp[name + '_ng'], f32)
        nb = np.asarray(inp[name + '_nb'], f32)
        W1 = np.asarray(inp[name + '_W1'], f32)
        b1 = np.asarray(inp[name + '_b1'], f32)
        b2 = np.asarray(inp[name + '_b2'], f32)
        b1e = b1 + nb @ W1.T
        if not (np.all(b1e == 0.0) and np.all(b2 == 0.0)):
            return None
        w1e.append(W1 * ng[None, :])
    if not (np.all(np.asarray(inp['ints_bh']) == 0.0)
            and np.all(np.asarray(inp['intl_bh']) == 0.0)):
        return None

    et = np.asarray(inp['evolved_tokens'], f32)
    Wp = np.asarray(inp['Wp'], f32)
    bp = np.asarray(inp['bp'], f32)
    proj = (et @ Wp.T + bp).astype(f32)
    tok3 = proj.reshape(B, K, D)

    vb = np.asarray(inp['var_batch']).astype(np.int64)
    aw = np.asarray(inp['attn_weights'], f32)
    zv0 = np.asarray(inp['z_var_0'], f32)
    vt = np.asarray(inp['var_types']).astype(f32)

    # global weight arrays
    W1eT = np.concatenate(w1e, axis=0).T.astype(f32)        # [256, 768]
    w1t_host = np.ascontiguousarray(
        W1eT.reshape(2, P, 768)).astype(bf16)
    w2t_host = np.zeros((2, P, 768), f32)
    for h, name in enumerate(('bin', 'ints', 'intl')):
        W2T = np.asarray(inp[name + '_W2'], f32).T          # [256, 256]
        for k in range(2):
            for j in range(2):
                w2t_host[k][:, h * 256 + j * P:h * 256 + (j + 1) * P] = \
                    W2T[k * P:(k + 1) * P, j * P:(j + 1) * P]
    w2t_host = w2t_host.astype(bf16)
    WheT = np.zeros((D, 16), f32)
    WheT[:, 0] = np.asarray(inp['bin_Wh'], f32)[0]
    WheT[:, 1:12] = np.asarray(inp['ints_Wh'], f32).T
    WheT[:, 12] = np.asarray(inp['intl_Wh'], f32)[0]
    wht_host = np.ascontiguousarray(WheT.reshape(2, P, 16)).astype(bf16)
    bhb_host = np.array([[0.5 * float(np.asarray(inp['bin_bh']).ravel()[0])]], f32)

    # per-core shards
    kb_needed = 2
    aug_all = []
    for c in range(NCORES):
        lo = c * NPC
        for g in range(NGRP):
            r0 = lo + g * TOK
            r1 = min(r0 + TOK, lo + NPC)
            if r0 >= r1:
                aug_all.append((c, g, None))
                continue
            bs = np.unique(vb[r0:r1])
            kb_needed = max(kb_needed, len(bs))
            aug_all.append((c, g, bs))
    if kb_needed > 4:
        return None
    kb32 = 32 * kb_needed

    in_maps = []
    for c in range(NCORES):
        lo = c * NPC
        in2 = np.zeros((NGRP, kb32, TOK + D), f32)
        for cc, g, bs in aug_all:
            if cc != c or bs is None:
                continue
            r0 = lo + g * TOK
            r1 = min(r0 + TOK, lo + NPC)
            avb = vb[r0:r1]
            a = aw[r0:r1]
            nv = r1 - r0
            cols = (np.arange(nv) % NSUB) * P + np.arange(nv) // NSUB
            for j, bb in enumerate(bs):
                m = (avb == bb).astype(f32)
                blockT = np.zeros((32, TOK), f32)
                blockT[:, cols] = (a * m[:, None]).T
                in2[g, j * 32:(j + 1) * 32, 0:TOK] = blockT
                in2[g, j * 32:(j + 1) * 32, TOK:] = tok3[bb]

        def padded(x):
            out = np.empty((NPAD,) + x.shape[1:], x.dtype)
            out[:NPC] = x[lo:lo + NPC]
            out[NPC:] = x[lo + NPC - 1]
            return out

        in_maps.append({
            "in2": in2.astype(ml_dtypes.bfloat16),
            "zv0": np.ascontiguousarray(padded(zv0)),
            "vt": _pack_cols(padded(vt)),
            "lb": _pack_cols(padded(lb)),
            "ub": _pack_cols(padded(ub)),
            "w1t": w1t_host,
            "w2t": w2t_host,
            "wht": wht_host,
            "bhb": bhb_host,
        })
    return in_maps, kb32


def _assemble_outputs(results):
    z_out = np.empty((N, D), np.float32)
    logits = np.empty((N, NC_CLS), np.float32)
    prob = np.empty((N,), np.float32)
    pred = np.empty((N,), np.float32)
    mb = np.empty((N,), np.float32)
    ms = np.empty((N,), np.float32)
    ml = np.empty((N,), np.float32)
    off = np.empty((N,), np.float32)
    rng = np.empty((N,), np.int32)
    for c, r in enumerate(results):
        sl = slice(c * NPC, (c + 1) * NPC)
        z_out[sl] = r["oz"][:NPC]
        logits[sl] = r["olog"].reshape(P, NGRP, NSUB, NC_CLS).transpose(
            1, 0, 2, 3).reshape(NPAD, NC_CLS)[:NPC]
        prob[sl] = _unpack_cols(r["oprob"])[:NPC]
        pred[sl] = _unpack_cols(r["opred"])[:NPC]
        mb[sl] = _unpack_cols(r["omb"])[:NPC]
        ms[sl] = _unpack_cols(r["oms"])[:NPC]
        ml[sl] = _unpack_cols(r["oml"])[:NPC]
        off[sl] = _unpack_cols(r["ooff"])[:NPC]
        rng[sl] = _unpack_cols(r["orng"])[:NPC]
    return (z_out, prob.reshape(N, 1), logits, pred.reshape(N, 1),
            mb != 0, ms != 0, ml != 0, off, rng.astype(np.int32))


def kernel(_trace=False, _use_tanh_for_sim=False, **inputs):
    inputs = {k: np.asarray(v) for k, v in inputs.items()}
    prep = _prep_inputs(inputs)
    if prep is None:
        return _host_fallback(inputs)
    in_maps, kb32 = prep
    nc = _get_program(kb32, _use_tanh_for_sim)
    res = run_bass_kernel_spmd(
        nc, in_maps, core_ids=list(range(NCORES)),
        trace=_trace, trace_cores=list(range(NCORES)) if _trace else None,
    )
    out = _assemble_outputs(res.results)
    if _trace:
        return out, res
    return out


# revision 19
# speedup vs baseline: 1.0755x; 1.0755x over previous
"""Trainium2 Bass kernel for nn_DeslicingDecoder (moe_routing).

Contract: kernel(**inputs) takes FULL unsharded numpy inputs (as produced by
setup_inputs()) and returns the FULL output tuple matching reference():
  (z_out[N,256] f32, prob_bin[N,1] f32, logits[N,11] f32, pred_large[N,1] f32,
   mask_bin[N] bool, mask_int_small[N] bool, mask_int_large[N] bool,
   offsets[N] f32, ranges[N] int32)

Sharding: N=30000 variables split contiguously across 8 NeuronCores (3750
rows each, padded to 4096).  The per-graph token tensor is replicated via a
host-precomputed projection; the deslice gather is turned into a dense
matmul per 256-token group against a per-group "augmented" attention matrix
(columns blocked by the <=KB batches the group spans), so all shapes are
static regardless of where batch boundaries fall.

Device pipeline per 256-token group (2 subtiles of 128 tokens):
  deslice matmul (fp32r) -> +z_var_0 (DVE) -> LayerNorm1 stats (bn_stats)
  -> batched Newton rsqrt (DVE) -> z_out (Pool) -> z_ln2 bf16 (Pool)
  -> DMA-transpose to feature-major -> W1 (bf16) -> Gelu (ACT, one op per
  layer over a 3-bank PSUM tile) -> W2 (bf16) -> Gelu -> head matmuls with
  the residual folded in via PSUM accumulation + per-token rescale (DVE)
  -> logit masking (DVE select).  Routing masks/offsets/ranges are computed
  once per core on [128,32]-packed tiles.
"""

import functools
from contextlib import ExitStack

import ml_dtypes
import numpy as np

import concourse.bass as bass
import concourse.tile as tile
from concourse import bacc, masks, mybir
from concourse.alu_op_type import AluOpType as ALU
from concourse.bass_utils import run_bass_kernel_spmd

F32 = mybir.dt.float32
F32R = mybir.dt.float32r
BF16 = mybir.dt.bfloat16
I32 = mybir.dt.int32
AF = mybir.ActivationFunctionType

# problem constants (hardcoded per contract)
N, K, B, D, T = 30000, 32, 8, 256, 10
NC_CLS = T + 1          # 11
LB_COL, UB_COL = 21, 22
INF = 1e18

NCORES = 8
NPC = N // NCORES       # 3750 rows per core
NPAD = 3840             # padded rows per core
TOK = 256               # tokens per group
NSUB = TOK // 128       # 2 subtiles per group
NGRP = NPAD // TOK      # 15 groups
NT = NPAD // 128        # 30 subtile columns

P = 128
HEAD_COLS = ((0, 1), (1, 12), (12, 13))   # bin, ints, intl columns in y


# --------------------------------------------------------------------------
# device program
# --------------------------------------------------------------------------

def _build_program(kb32: int, use_tanh_for_sim: bool = False):
    """Build the Bass/Tile program.  kb32 = 32 * max batches per group."""
    gelu_f = AF.Tanh if use_tanh_for_sim else AF.Gelu

    nc = bacc.Bacc("TRN2", target_bir_lowering=False, debug=False)

    # ---- dram io
    in2_d = nc.dram_tensor("in2", [NGRP, kb32, TOK + D], BF16, kind="ExternalInput").ap()
    zv0_d = nc.dram_tensor("zv0", [NPAD, D], F32, kind="ExternalInput").ap()
    vt_d = nc.dram_tensor("vt", [P, NT], F32, kind="ExternalInput").ap()
    lb_d = nc.dram_tensor("lb", [P, NT], F32, kind="ExternalInput").ap()
    ub_d = nc.dram_tensor("ub", [P, NT], F32, kind="ExternalInput").ap()
    w1_d = nc.dram_tensor("w1t", [2, P, 768], BF16, kind="ExternalInput").ap()
    w2_d = nc.dram_tensor("w2t", [2, P, 768], BF16, kind="ExternalInput").ap()
    wh_d = nc.dram_tensor("wht", [2, P, 16], BF16, kind="ExternalInput").ap()
    bhb_d = nc.dram_tensor("bhb", [1, 1], F32, kind="ExternalInput").ap()  # 0.5*bh_bin

    oz_d = nc.dram_tensor("oz", [NPAD, D], F32, kind="ExternalOutput").ap()
    olog_d = nc.dram_tensor("olog", [P, NT * NC_CLS], F32, kind="ExternalOutput").ap()
    oprob_d = nc.dram_tensor("oprob", [P, NT], F32, kind="ExternalOutput").ap()
    opred_d = nc.dram_tensor("opred", [P, NT], F32, kind="ExternalOutput").ap()
    omb_d = nc.dram_tensor("omb", [P, NT], F32, kind="ExternalOutput").ap()
    oms_d = nc.dram_tensor("oms", [P, NT], F32, kind="ExternalOutput").ap()
    oml_d = nc.dram_tensor("oml", [P, NT], F32, kind="ExternalOutput").ap()
    ooff_d = nc.dram_tensor("ooff", [P, NT], F32, kind="ExternalOutput").ap()
    orng_d = nc.dram_tensor("orng", [P, NT], I32, kind="ExternalOutput").ap()

    with tile.TileContext(nc) as tc, ExitStack() as ctx:
        const = ctx.enter_context(tc.tile_pool(name="const", bufs=1))
        io = ctx.enter_context(tc.tile_pool(name="io", bufs=4))
        zw = ctx.enter_context(tc.tile_pool(name="zw", bufs=NGRP))
        zo = ctx.enter_context(tc.tile_pool(name="zo", bufs=3))
        zl = ctx.enter_context(tc.tile_pool(name="zl", bufs=3))
        gp = ctx.enter_context(tc.tile_pool(name="gp", bufs=2))
        stp = ctx.enter_context(tc.tile_pool(name="stp", bufs=3))
        rt = ctx.enter_context(tc.tile_pool(name="rt", bufs=1))
        ps_small = ctx.enter_context(tc.tile_pool(name="ps_small", bufs=2, space="PSUM"))
        ps_h = ctx.enter_context(tc.tile_pool(name="ps_h", bufs=2, space="PSUM"))

        # ---- constants / weights
        w1t = const.tile([P, 2, 768], BF16)
        nc.sync.dma_start(w1t, w1_d.rearrange("k p f -> p k f"))
        w2t = const.tile([P, 2, 768], BF16)
        nc.sync.dma_start(w2t, w2_d.rearrange("k p f -> p k f"))
        wht = const.tile([P, 2, 16], BF16)
        nc.sync.dma_start(wht, wh_d.rearrange("k p f -> p k f"))
        bhb = const.tile([P, 1], F32)
        nc.sync.dma_start(bhb, bhb_d.to_broadcast((P, 1)))

        idb = const.tile([P, P], BF16)
        masks.make_identity(nc, idb)

        iota_i = const.tile([P, NC_CLS], I32)
        nc.gpsimd.iota(iota_i, [[1, NC_CLS]], channel_multiplier=0)
        iota_f = const.tile([P, NC_CLS], F32)
        nc.vector.tensor_copy(iota_f, iota_i)
        neg9 = const.tile([P, NC_CLS], F32)
        nc.vector.memset(neg9, -1e9)

        # ---- routing (once per core, [128, NT] col-major packed)
        vt_t = rt.tile([P, NT], F32)
        nc.sync.dma_start(vt_t, vt_d)
        lb_t = rt.tile([P, NT], F32)
        nc.sync.dma_start(lb_t, lb_d)
        ub_t = rt.tile([P, NT], F32)
        nc.sync.dma_start(ub_t, ub_d)

        mb = rt.tile([P, NT], F32)
        nc.vector.tensor_scalar(mb, vt_t, 1.0, None, ALU.is_equal)
        nc.sync.dma_start(omb_d, mb)
        isint = rt.tile([P, NT], F32)
        nc.vector.tensor_scalar(isint, vt_t, 2.0, None, ALU.is_equal)
        fin = rt.tile([P, NT], F32)
        nc.vector.tensor_tensor(fin, lb_t, lb_t, ALU.mult)
        nc.vector.tensor_scalar(fin, fin, INF * INF, None, ALU.is_lt)
        fu = rt.tile([P, NT], F32)
        nc.vector.tensor_tensor(fu, ub_t, ub_t, ALU.mult)
        nc.vector.tensor_scalar(fu, fu, INF * INF, None, ALU.is_lt)
        nc.vector.tensor_tensor(fin, fin, fu, ALU.mult)
        dd = rt.tile([P, NT], F32)
        nc.vector.tensor_tensor(dd, ub_t, lb_t, ALU.subtract)
        sle = rt.tile([P, NT], F32)
        nc.vector.tensor_scalar(sle, dd, float(T), None, ALU.is_le)
        sm = rt.tile([P, NT], F32)
        nc.vector.tensor_tensor(sm, isint, fin, ALU.mult)
        nc.vector.tensor_tensor(sm, sm, sle, ALU.mult)
        nc.sync.dma_start(oms_d, sm)
        lg = rt.tile([P, NT], F32)
        nc.vector.tensor_tensor(lg, isint, sm, ALU.subtract)
        nc.sync.dma_start(oml_d, lg)
        nc.sync.dma_start(ooff_d, lb_t)     # offsets = floor(lb) = lb (integral)
        rngf = rt.tile([P, NT], F32)
        nc.vector.tensor_scalar(rngf, dd, 1.0, float(NC_CLS), ALU.add, ALU.min)
        nc.vector.tensor_scalar(rngf, rngf, 1.0, None, ALU.max)
        rngi = rt.tile([P, NT], I32)
        nc.vector.tensor_copy(rngi, rngf)
        nc.sync.dma_start(orng_d, rngi)

        y_all = rt.tile([P, NT, 16], F32)
        logits_buf = rt.tile([P, NT, NC_CLS], F32)
        msk3 = rt.tile([P, NT, NC_CLS], mybir.dt.uint8)
        i_b = bass.AP(tensor=iota_f.tensor, offset=iota_f.offset,
                      ap=[iota_f.ap[0], [0, NT], [1, NC_CLS]])
        r_b = bass.AP(tensor=rngf.tensor, offset=rngf.offset,
                      ap=[rngf.ap[0], [1, NT], [0, NC_CLS]])
        nc.vector.tensor_tensor(msk3, i_b, r_b, ALU.is_lt)

        # ---- phase A: deslice + z_sum + LN1 stats
        mv = rt.tile([P, NT, 2], F32)
        zs_list = [None] * NGRP

        def phase_a(g):
            in2 = io.tile([kb32, TOK + D], BF16)
            nc.scalar.dma_start(in2, in2_d[g])
            zv0 = io.tile([P, NSUB, D], F32)
            nc.scalar.dma_start(
                zv0, zv0_d[g * TOK:(g + 1) * TOK, :].rearrange(
                    "(p s) d -> p s d", s=NSUB))
            zdes = ps_small.tile([P, NSUB, D], F32, tag="sp")
            attn_ap = in2[:, 0:TOK]
            tok_ap = in2[:, TOK:TOK + D]
            for s in range(NSUB):
                nc.tensor.matmul(
                    zdes[:, s, :], attn_ap[:, s * P:(s + 1) * P], tok_ap,
                    start=True, stop=True)
            zs = zw.tile([P, NSUB, D], F32)
            nc.vector.tensor_tensor(zs, zdes, zv0, ALU.add)
            st = stp.tile([P, NSUB, 6], F32)
            for s in range(NSUB):
                nc.vector.bn_stats(st[:, s, :], zs[:, s, :])
                nc.vector.bn_aggr(mv[:, g * NSUB + s, :], st[:, s, :])
            zs_list[g] = zs

        # Newton rsqrt over a column range of mv:
        # y1 = rsqrt(var1+eps); q = rsqrt(var2+eps), var2 = var1*y1^2;
        # inv2 = 1/q; rc = y1*q
        y1 = rt.tile([P, NT], F32)
        q = rt.tile([P, NT], F32)
        inv2 = rt.tile([P, NT], F32)
        rc = rt.tile([P, NT], F32)

        def newton(c0, c1):
            w = c1 - c0
            vv = stp.tile([P, NT], F32, tag="vv")
            vvs = vv[:, 0:w]
            nc.vector.tensor_scalar(vvs, mv[:, c0:c1, 1], 1e-5, None, ALU.add)
            y1s = y1[:, c0:c1]
            nc.vector.memset(y1s, 0.7)
            tq = stp.tile([P, NT], F32, tag="tq")
            tqs = tq[:, 0:w]
            for _ in range(4):
                nc.vector.tensor_tensor(tqs, y1s, y1s, ALU.mult)
                nc.vector.tensor_tensor(tqs, tqs, vvs, ALU.mult)
                nc.vector.tensor_scalar(tqs, tqs, -0.5, 1.5, ALU.mult, ALU.add)
                nc.vector.tensor_tensor(y1s, y1s, tqs, ALU.mult)
            v2p = stp.tile([P, NT], F32, tag="v2p")
            v2s = v2p[:, 0:w]
            nc.vector.tensor_tensor(v2s, y1s, y1s, ALU.mult)
            nc.vector.tensor_tensor(v2s, v2s, mv[:, c0:c1, 1], ALU.mult)
            nc.vector.tensor_scalar(v2s, v2s, 1e-5, None, ALU.add)
            qs = q[:, c0:c1]
            nc.vector.memset(qs, 1.0)
            for _ in range(2):
                nc.vector.tensor_tensor(tqs, qs, qs, ALU.mult)
                nc.vector.tensor_tensor(tqs, tqs, v2s, ALU.mult)
                nc.vector.tensor_scalar(tqs, tqs, -0.5, 1.5, ALU.mult, ALU.add)
                nc.vector.tensor_tensor(qs, qs, tqs, ALU.mult)
            nc.vector.tensor_tensor(inv2[:, c0:c1], v2s, qs, ALU.mult)
            nc.vector.tensor_tensor(rc[:, c0:c1], y1s, qs, ALU.mult)

        # ---- phase B: LN apply + MLP heads per group
        def bc2(tile2d, g, width):
            # [128, 2] slice (cols 2g, 2g+1) broadcast along a width-free dim
            return bass.AP(tensor=tile2d.tensor, offset=tile2d.offset + 2 * g,
                           ap=[tile2d.ap[0], [1, 2], [0, width]])

        def phase_b(g):
            zs = zs_list[g]
            tln = zl.tile([P, NSUB, D], F32, tag="tln")
            m_bc = bass.AP(tensor=mv.tensor, offset=mv.offset + 4 * g,
                           ap=[mv.ap[0], [2, 2], [0, D]])
            nc.vector.tensor_tensor(tln, zs, m_bc, ALU.subtract)
            zout = zo.tile([P, NSUB, D], F32)
            nc.vector.tensor_tensor(zout, tln, bc2(y1, g, D), ALU.mult)
            zln2 = zl.tile([P, NSUB, D], BF16)
            nc.vector.tensor_tensor(zln2, tln, bc2(rc, g, D), ALU.mult)
            nc.scalar.dma_start(
                oz_d[g * TOK:(g + 1) * TOK, :].rearrange(
                    "(p s) d -> p s d", s=NSUB), zout)

            # transpose z_ln2 to feature-major via PE + one DVE copy
            zt_ps = ps_small.tile([P, 2, NSUB, P], BF16, tag="sp")
            for k in range(2):
                for s_i in range(NSUB):
                    nc.tensor.transpose(
                        zt_ps[:, k, s_i, :], zln2[:, s_i, k * P:(k + 1) * P], idb)
            zts = gp.tile([P, 2, NSUB, P], BF16, tag="zts")
            nc.scalar.copy(zts, zt_ps)

            # W1: feature-major h1T [6*128 hid, TOK]
            h1 = ps_h.tile([P, 6, TOK], F32, tag="h")
            for c in range(6):
                for k in range(2):
                    nc.tensor.matmul(
                        h1[:, c, :],
                        w1t[:, k, c * P:(c + 1) * P],
                        zts[:, k, :, :],
                        start=(k == 0), stop=(k == 1))
            g1 = gp.tile([P, 6, TOK], BF16)
            nc.scalar.activation(g1, h1, gelu_f)

            # W2 per head: h2T chunks (head h, dout chunk j)
            h2 = ps_h.tile([P, 6, TOK], F32, tag="h")
            for h in range(3):
                for j in range(2):
                    for k in range(2):
                        nc.tensor.matmul(
                            h2[:, h * 2 + j, :],
                            w2t[:, k, h * 256 + j * P:h * 256 + (j + 1) * P],
                            g1[:, h * 2 + k, :],
                            start=(k == 0), stop=(k == 1))
            g2 = gp.tile([P, 6, TOK], BF16)
            nc.scalar.activation(g2, h2, gelu_f)

            # heads: y = inv2*(z_ln2 @ WheT) + g2 @ WheT   (residual folded)
            yps = ps_small.tile([P, NSUB, 2, 16], F32, tag="sp")
            for s_i in range(NSUB):
                for k in range(2):
                    nc.tensor.matmul(
                        yps[:, s_i, 0, 0:13], zts[:, k, s_i, :], wht[:, k, 0:13],
                        start=(k == 0), stop=(k == 1))
                for h, (o0, o1) in enumerate(HEAD_COLS):
                    for k in range(2):
                        nc.tensor.matmul(
                            yps[:, s_i, 1, o0:o1],
                            g2[:, h * 2 + k, s_i * P:(s_i + 1) * P],
                            wht[:, k, o0:o1],
                            start=(k == 0), stop=(k == 1))
            ya = yps[:, :, 0, 0:13]
            yb = yps[:, :, 1, 0:13]
            ysl = bass.AP(tensor=y_all.tensor, offset=y_all.offset + 32 * g,
                          ap=[y_all.ap[0], [16, 2], [1, 13]])
            nc.vector.tensor_tensor(ysl, ya, bc2(inv2, g, 13), ALU.mult)
            nc.vector.tensor_tensor(ysl, ysl, yb, ALU.add)
            for s_i in range(NSUB):
                c = g * NSUB + s_i
                nc.vector.select(
                    logits_buf[:, c, :], msk3[:, c, :],
                    y_all[:, c, 1:12], neg9)


        HALF = (NGRP + 1) // 2
        for g in range(HALF):
            phase_a(g)
        newton(0, HALF * NSUB)
        for g in range(HALF, NGRP):
            phase_a(g)
        for g in range(HALF):
            phase_b(g)
        newton(HALF * NSUB, NT)
        for g in range(HALF, NGRP):
            phase_b(g)

        # ---- final outputs
        nc.sync.dma_start(
            olog_d.rearrange("p (j o) -> p j o", o=NC_CLS), logits_buf)
        tanh_t = rt.tile([P, NT], F32)
        nc.scalar.activation(tanh_t, y_all[:, :, 0], AF.Tanh, bias=bhb, scale=0.5)
        prob_t = rt.tile([P, NT], F32)
        nc.vector.tensor_scalar(prob_t, tanh_t, 0.5, 0.5, ALU.mult, ALU.add)
        nc.sync.dma_start(oprob_d, prob_t)
        nc.sync.dma_start(opred_d, y_all[:, :, 12])

    nc.compile()
    return nc


@functools.lru_cache(maxsize=4)
def _get_program(kb32: int, use_tanh_for_sim: bool = False):
    return _build_program(kb32, use_tanh_for_sim)


# --------------------------------------------------------------------------
# host side
# --------------------------------------------------------------------------

def _np_gelu(x):
    from scipy.special import erf
    return 0.5 * x * (1.0 + erf(x / np.sqrt(2.0).astype(np.float32)))


def _np_ln(x, g, b, eps=1e-5):
    m = x.mean(-1, keepdims=True)
    v = ((x - m) ** 2).mean(-1, keepdims=True)
    return (x - m) / np.sqrt(v + eps) * g + b


def _np_head(x, ng, nb, W1, b1, W2, b2, Wh, bh):
    h = _np_ln(x, ng, nb)
    h = _np_gelu(h @ W1.T + b1)
    h = _np_gelu(h @ W2.T + b2)
    return (x + h) @ Wh.T + bh


def _host_fallback(inp):
    """Pure numpy reference path (used only for inputs outside the fast
    path's host-verified assumptions)."""
    et = inp['evolved_tokens'].astype(np.float32)
    proj = et @ inp['Wp'].T.astype(np.float32) + inp['bp']
    tok3 = proj.reshape(B, K, D)
    vb = np.asarray(inp['var_batch']).astype(np.int64)
    aw = inp['attn_weights'].astype(np.float32)
    z_des = np.einsum('nk,nkd->nd', aw, tok3[vb]).astype(np.float32)
    z_out = _np_ln(z_des + inp['z_var_0'], inp['fn_g'], inp['fn_b']).astype(np.float32)
    vf = inp['variable_features']
    lb = vf[:, LB_COL]
    ub = vf[:, UB_COL]
    vt = np.asarray(inp['var_types'])
    is_int = vt == 2
    finite = (np.abs(lb) < INF) & (np.abs(ub) < INF)
    m_small = is_int & finite & ((ub - lb) <= T)
    m_large = is_int & ~m_small
    m_bin = vt == 1
    offsets = np.floor(lb).astype(np.float32)
    ranges = np.clip(np.ceil(ub) - offsets + 1.0, 1.0, NC_CLS).astype(np.int32)
    prob_bin = 1.0 / (1.0 + np.exp(-_np_head(
        z_out, inp['bin_ng'], inp['bin_nb'], inp['bin_W1'], inp['bin_b1'],
        inp['bin_W2'], inp['bin_b2'], inp['bin_Wh'], inp['bin_bh'])))
    logits = _np_head(
        z_out, inp['ints_ng'], inp['ints_nb'], inp['ints_W1'], inp['ints_b1'],
        inp['ints_W2'], inp['ints_b2'], inp['ints_Wh'], inp['ints_bh'])
    valid = np.arange(NC_CLS)[None, :] < ranges[:, None]
    logits = np.where(valid, logits, np.float32(-1e9)).astype(np.float32)
    pred_large = _np_head(
        z_out, inp['intl_ng'], inp['intl_nb'], inp['intl_W1'], inp['intl_b1'],
        inp['intl_W2'], inp['intl_b2'], inp['intl_Wh'], inp['intl_bh'])
    return (z_out.astype(np.float32), prob_bin.astype(np.float32),
            logits, pred_large.astype(np.float32),
            m_bin, m_small, m_large, offsets, ranges)


def _pack_cols(x):
    """[NPAD] -> [128, NT]; packed[p, 2g+s] = x[g*256 + 2p + s]."""
    return np.ascontiguousarray(
        x.reshape(NGRP, P, NSUB).transpose(1, 0, 2).reshape(P, NT))


def _unpack_cols(x):
    return np.ascontiguousarray(
        x.reshape(P, NGRP, NSUB).transpose(1, 0, 2).reshape(-1))


def _prep_inputs(inp):
    """Build per-core device input maps.  Returns (in_maps, kb32) or None if
    the inputs fall outside the fast path's assumptions."""
    f32 = np.float32
    bf16 = ml_dtypes.bfloat16

    fn_g = np.asarray(inp['fn_g'], f32)
    fn_b = np.asarray(inp['fn_b'], f32)
    if not (np.all(fn_g == 1.0) and np.all(fn_b == 0.0)):
        return None
    vf = np.asarray(inp['variable_features'], f32)
    lb = vf[:, LB_COL].astype(f32)
    ub = vf[:, UB_COL].astype(f32)
    if not (np.all(np.floor(lb) == lb) and np.all(np.ceil(ub) == ub)):
        return None

    # fold per-head norm gains into W1, check biases are zero
    w1e = []
    for name in ('bin', 'ints', 'intl'):
        ng = np.asarray(inp[name + '_ng'], f32)
        nb = np.asarray(inp[name + '_nb'], f32)
        W1 = np.asarray(inp[name + '_W1'], f32)
        b1 = np.asarray(inp[name + '_b1'], f32)
        b2 = np.asarray(inp[name + '_b2'], f32)
        b1e = b1 + nb @ W1.T
        if not (np.all(b1e == 0.0) and np.all(b2 == 0.0)):
            return None
        w1e.append(W1 * ng[None, :])
    if not (np.all(np.asarray(inp['ints_bh']) == 0.0)
            and np.all(np.asarray(inp['intl_bh']) == 0.0)):
        return None

    et = np.asarray(inp['evolved_tokens'], f32)
    Wp = np.asarray(inp['Wp'], f32)
    bp = np.asarray(inp['bp'], f32)
    proj = (et @ Wp.T + bp).astype(f32)
    tok3 = proj.reshape(B, K, D)

    vb = np.asarray(inp['var_batch']).astype(np.int64)
    aw = np.asarray(inp['attn_weights'], f32)
    zv0 = np.asarray(inp['z_var_0'], f32)
    vt = np.asarray(inp['var_types']).astype(f32)

    # global weight arrays
    W1eT = np.concatenate(w1e, axis=0).T.astype(f32)        # [256, 768]
    w1t_host = np.ascontiguousarray(
        W1eT.reshape(2, P, 768)).astype(bf16)
    w2t_host = np.zeros((2, P, 768), f32)
    for h, name in enumerate(('bin', 'ints', 'intl')):
        W2T = np.asarray(inp[name + '_W2'], f32).T          # [256, 256]
        for k in range(2):
            for j in range(2):
                w2t_host[k][:, h * 256 + j * P:h * 256 + (j + 1) * P] = \
                    W2T[k * P:(k + 1) * P, j * P:(j + 1) * P]
    w2t_host = w2t_host.astype(bf16)
    WheT = np.zeros((D, 16), f32)
    WheT[:, 0] = np.asarray(inp['bin_Wh'], f32)[0]
    WheT[:, 1:12] = np.asarray(inp['ints_Wh'], f32).T
    WheT[:, 12] = np.asarray(inp['intl_Wh'], f32)[0]
    wht_host = np.ascontiguousarray(WheT.reshape(2, P, 16)).astype(bf16)
    bhb_host = np.array([[0.5 * float(np.asarray(inp['bin_bh']).ravel()[0])]], f32)

    # per-core shards
    kb_needed = 2
    aug_all = []
    for c in range(NCORES):
        lo = c * NPC
        for g in range(NGRP):
            r0 = lo + g * TOK
            r1 = min(r0 + TOK, lo + NPC)
            if r0 >= r1:
                aug_all.append((c, g, None))
                continue
            bs = np.unique(vb[r0:r1])
            kb_needed = max(kb_needed, len(bs))
            aug_all.append((c, g, bs))
    if kb_needed > 4:
        return None
    kb32 = 32 * kb_needed

    in_maps = []
    for c in range(NCORES):
        lo = c * NPC
        in2 = np.zeros((NGRP, kb32, TOK + D), f32)
        for cc, g, bs in aug_all:
            if cc != c or bs is None:
                continue
            r0 = lo + g * TOK
            r1 = min(r0 + TOK, lo + NPC)
            avb = vb[r0:r1]
            a = aw[r0:r1]
            nv = r1 - r0
            cols = (np.arange(nv) % NSUB) * P + np.arange(nv) // NSUB
            for j, bb in enumerate(bs):
                m = (avb == bb).astype(f32)
                blockT = np.zeros((32, TOK), f32)
                blockT[:, cols] = (a * m[:, None]).T
                in2[g, j * 32:(j + 1) * 32, 0:TOK] = blockT
                in2[g, j * 32:(j + 1) * 32, TOK:] = tok3[bb]

        def padded(x):
            out = np.empty((NPAD,) + x.shape[1:], x.dtype)
            out[:NPC] = x[lo:lo + NPC]
            out[NPC:] = x[lo + NPC - 1]
            return out

        in_maps.append({
            "in2": in2.astype(ml_dtypes.bfloat16),
            "zv0": np.ascontiguousarray(padded(zv0)),
            "vt": _pack_cols(padded(vt)),
            "lb": _pack_cols(padded(lb)),
            "ub": _pack_cols(padded(ub)),
            "w1t": w1t_host,
            "w2t": w2t_host,
            "wht": wht_host,
            "bhb": bhb_host,
        })
    return in_maps, kb32


def _assemble_outputs(results):
    z_out = np.empty((N, D), np.float32)
    logits = np.empty((N, NC_CLS), np.float32)
    prob = np.empty((N,), np.float32)
    pred = np.empty((N,), np.float32)
    mb = np.empty((N,), np.float32)
    ms = np.empty((N,), np.float32)
    ml = np.empty((N,), np.float32)
    off = np.empty((N,), np.float32)
    rng = np.empty((N,), np.int32)
    for c, r in enumerate(results):
        sl = slice(c * NPC, (c + 1) * NPC)
        z_out[sl] = r["oz"][:NPC]
        logits[sl] = r["olog"].reshape(P, NGRP, NSUB, NC_CLS).transpose(
            1, 0, 2, 3).reshape(NPAD, NC_CLS)[:NPC]
        prob[sl] = _unpack_cols(r["oprob"])[:NPC]
        pred[sl] = _unpack_cols(r["opred"])[:NPC]
        mb[sl] = _unpack_cols(r["omb"])[:NPC]
        ms[sl] = _unpack_cols(r["oms"])[:NPC]
        ml[sl] = _unpack_cols(r["oml"])[:NPC]
        off[sl] = _unpack_cols(r["ooff"])[:NPC]
        rng[sl] = _unpack_cols(r["orng"])[:NPC]
    return (z_out, prob.reshape(N, 1), logits, pred.reshape(N, 1),
            mb != 0, ms != 0, ml != 0, off, rng.astype(np.int32))


def kernel(_trace=False, _use_tanh_for_sim=False, **inputs):
    inputs = {k: np.asarray(v) for k, v in inputs.items()}
    prep = _prep_inputs(inputs)
    if prep is None:
        return _host_fallback(inputs)
    in_maps, kb32 = prep
    nc = _get_program(kb32, _use_tanh_for_sim)
    res = run_bass_kernel_spmd(
        nc, in_maps, core_ids=list(range(NCORES)),
        trace=_trace, trace_cores=list(range(NCORES)) if _trace else None,
    )
    out = _assemble_outputs(res.results)
    if _trace:
        return out, res
    return out
